# revision 4
# baseline (speedup 1.0000x reference)
"""Trainium2 Bass kernel for nn_GSCAN_model (gnn_message_passing).

Reference computation (per cell of a [B, 32, 32, 17] grid):
    emb    = concat(x[0:4] @ W_size, x[4:8] @ W_shape,
                    x[8:12] @ W_rgb, x[12:17] @ W_agent)     # [64]
    mask   = sum(x) > 0
    out    = mask ? emb : [x, zeros(47)]                     # [64]

Kernel formulation: fold the mask into the matmul.
    out = (x*m) @ (Wblk - P17)  +  pad(x)
where Wblk is the 17x64 block-diagonal assembly of the four small
weights.  Equivalently  out = (x*m) @ (Wblk - P17) + pad(x): one fp32
tensor-engine matmul per cell group plus a GPSIMD add of raw x on the
first 17 channels.  Everything stays fp32 (max abs err ~1e-6).

The matmul needs channels-on-partitions, the output DMA needs
cells-on-partitions.  Per macro tile (128 partitions x 32 cells) we
batch 7 cell-subtiles per PE transpose ([128, 119] -> [119, 128]) and
run one matmul per group against a host-built block-diagonal weight
Wd [119, 448] whose result lands cells-on-partitions, giving 2 KB
contiguous output DMA per partition.

Data parallel over 8 NeuronCores: batch dim 2048 -> 256 per core.
"""

import numpy as np

B, H, W, C_IN = 2048, 32, 32, 17
EMB = 64
N_CORES = 8
P = 128                      # partitions
C_SLOTS = 32                 # cells per partition per macro tile
CELLS_PER_CORE = (B // N_CORES) * H * W          # 262144
MACROS = CELLS_PER_CORE // (P * C_SLOTS)         # 64
# groups of subtiles per macro: 4 groups of 7 slots + 1 group of 4 slots
GROUPS = [(0, 7), (7, 7), (14, 7), (21, 7), (28, 4)]

_CACHE = {}


def _build_program(n_macros):
    import concourse.bacc as bacc
    import concourse.mybir as mybir
    from concourse.tile import TileContext

    f32 = mybir.dt.float32
    f32r = mybir.dt.float32r
    nc = bacc.Bacc("TRN2", target_bir_lowering=False, debug=False,
                   num_devices=N_CORES)

    cells = n_macros * P * C_SLOTS
    x = nc.dram_tensor("x", [cells, C_IN], f32, kind="ExternalInput")
    wd = nc.dram_tensor("wd", [7 * C_IN, 7 * EMB], f32, kind="ExternalInput")
    ident = nc.dram_tensor("ident", [P, P], f32, kind="ExternalInput")
    y = nc.dram_tensor("y", [cells, EMB], f32, kind="ExternalOutput")

    xr = x.ap().rearrange("(m p c) k -> m p (c k)", p=P, c=C_SLOTS)
    yr = y.ap().rearrange("(m p c) n -> m p (c n)", p=P, c=C_SLOTS)

    KMAX = 7 * C_IN              # 119 rows: largest group
    NMAX = 7 * EMB               # 448 cols

    with TileContext(nc) as tc:
        with (
            tc.tile_pool(name="const", bufs=1) as constp,
            tc.tile_pool(name="xin", bufs=6) as xin_pool,
            tc.tile_pool(name="mask", bufs=8) as mask_pool,
            tc.tile_pool(name="xm", bufs=6) as xm_pool,
            tc.tile_pool(name="xat", bufs=4) as xat_pool,
            tc.tile_pool(name="outp", bufs=6) as out_pool,
            tc.tile_pool(name="pstA", bufs=2, space="PSUM") as pstA_pool,
            tc.tile_pool(name="pstB", bufs=2, space="PSUM") as pstB_pool,
            tc.tile_pool(name="pso", bufs=4, space="PSUM") as pso_pool,
        ):
            wd_t = constp.tile([KMAX, NMAX], f32)
            nc.sync.dma_start(out=wd_t, in_=wd.ap())
            id_t = constp.tile([P, P], f32)
            nc.sync.dma_start(out=id_t, in_=ident.ap())

            for mi in range(n_macros):
                xt = xin_pool.tile([P, C_SLOTS * C_IN], f32)
                nc.sync.dma_start(out=xt, in_=xr[mi])
                xt3 = xt.rearrange("p (c k) -> p c k", k=C_IN)

                s_t = mask_pool.tile([P, C_SLOTS], f32, tag="s")
                m_t = mask_pool.tile([P, C_SLOTS], f32, tag="m")
                nc.vector.tensor_reduce(out=s_t, in_=xt3,
                                        axis=mybir.AxisListType.X,
                                        op=mybir.AluOpType.add)
                nc.vector.tensor_scalar(out=m_t, in0=s_t, scalar1=0.0,
                                        scalar2=None,
                                        op0=mybir.AluOpType.is_gt)

                # xm = X * mask; with Wd built from (Wblk - P17) the
                # matmul yields m*emb - m*pad(X), and adding raw X on the
                # first 17 channels gives where(m, emb, pad(X)).
                xm = xm_pool.tile([P, C_SLOTS * C_IN], f32)
                xm3 = xm.rearrange("p (c k) -> p c k", k=C_IN)
                m_b = m_t.unsqueeze(2).broadcast_to((P, C_SLOTS, C_IN))
                nc.vector.tensor_tensor(out=xm3, in0=xt3, in1=m_b,
                                        op=mybir.AluOpType.mult)

                # 5 PE transposes: groups 0-3 -> psum bank A, group 4 -> B
                tpA = pstA_pool.tile([P, 4 * P], f32, tag="tpA")
                tpB = pstB_pool.tile([P, P], f32, tag="tpB")
                for gi, (c0, ns) in enumerate(GROUPS):
                    k = ns * C_IN
                    dst = (tpA[0:k, gi * P:(gi + 1) * P] if gi < 4
                           else tpB[0:k, :])
                    nc.tensor.transpose(
                        out=dst,
                        in_=xm[:, c0 * C_IN:(c0 + ns) * C_IN].bitcast(f32r),
                        identity=id_t.bitcast(f32r))
                xatA = xat_pool.tile([P, 4 * P], f32, tag="xatA")
                xatB = xat_pool.tile([P, P], f32, tag="xatB")
                nc.scalar.copy(out=xatA[0:KMAX, :], in_=tpA[0:KMAX, :])
                nc.vector.tensor_copy(out=xatB[0:4 * C_IN, :],
                                      in_=tpB[0:4 * C_IN, :])

                # 5 matmuls -> psum -> SBUF out tile
                out_t = out_pool.tile([P, C_SLOTS * EMB], f32)
                for gi, (c0, ns) in enumerate(GROUPS):
                    k = ns * C_IN
                    n = ns * EMB
                    lhsT = (xatA[0:k, gi * P:(gi + 1) * P] if gi < 4
                            else xatB[0:k, :])
                    po = pso_pool.tile([P, NMAX], f32, tag="po")
                    nc.tensor.matmul(out=po[:, 0:n], lhsT=lhsT.bitcast(f32r),
                                     rhs=wd_t[0:k, 0:n].bitcast(f32r),
                                     start=True, stop=True)
                    if gi == 4:
                        nc.vector.tensor_copy(
                            out=out_t[:, c0 * EMB:c0 * EMB + n],
                            in_=po[:, 0:n])
                    else:
                        nc.scalar.copy(out=out_t[:, c0 * EMB:c0 * EMB + n],
                                       in_=po[:, 0:n])

                # pad path: out[:, :, 0:17] += X.  Deliberately on GPSIMD:
                # this op must wait for all five PSUM->SBUF copies, and
                # GPSIMD is the only engine with no other duties -- putting
                # it (or the compares) on a busy engine measurably stalls
                # the pipeline (+50..130us in A/B runs).
                out3 = out_t.rearrange("p (c k) -> p c k", k=EMB)
                nc.gpsimd.tensor_tensor(out=out3[:, :, 0:C_IN],
                                        in0=out3[:, :, 0:C_IN], in1=xt3,
                                        op=mybir.AluOpType.add)

                nc.sync.dma_start(out=yr[mi], in_=out_t)
    nc.compile()
    return nc


def _host_weights(W_size, W_shape, W_rgb, W_agent):
    """Wd [119, 448]: 7 diagonal blocks of (Wblk - P17) [17, 64].

    Per slot the kernel feeds X*m; (X*m) @ (Wblk - P17) + X equals
    where(m, emb, pad(X)) -- the +X on channels 0:17 is applied by
    GPSIMD after the matmul.
    """
    wblk = np.zeros((C_IN, EMB), np.float32)
    wblk[0:4, 0:16] = W_size
    wblk[4:8, 16:32] = W_shape
    wblk[8:12, 32:48] = W_rgb
    wblk[12:17, 48:64] = W_agent
    pad = np.zeros((C_IN, EMB), np.float32)
    pad[np.arange(C_IN), np.arange(C_IN)] = 1.0
    w17 = wblk - pad                                 # [17, 64]
    wd = np.zeros((7 * C_IN, 7 * EMB), np.float32)
    for i in range(7):
        wd[i * C_IN:(i + 1) * C_IN, i * EMB:(i + 1) * EMB] = w17
    return wd


def kernel(situation, W_size, W_shape, W_rgb, W_agent):
    from concourse.bass_utils import run_bass_kernel_spmd

    key = "prog"
    if key not in _CACHE:
        _CACHE[key] = _build_program(MACROS)
    nc = _CACHE[key]

    wd = _host_weights(np.asarray(W_size), np.asarray(W_shape),
                       np.asarray(W_rgb), np.asarray(W_agent))
    ident = np.eye(P, dtype=np.float32)

    sit = np.ascontiguousarray(np.asarray(situation), dtype=np.float32)
    bpc = B // N_CORES
    in_maps = []
    for i in range(N_CORES):
        shard = sit[i * bpc:(i + 1) * bpc].reshape(CELLS_PER_CORE, C_IN)
        in_maps.append({"x": np.ascontiguousarray(shard),
                        "wd": wd, "ident": ident})

    res = run_bass_kernel_spmd(nc, in_maps, core_ids=list(range(N_CORES)))
    out = np.empty((B, H, W, EMB), np.float32)
    for i in range(N_CORES):
        out[i * bpc:(i + 1) * bpc] = res.results[i]["y"].reshape(
            bpc, H, W, EMB)
    return out

